# revision 1
# baseline (speedup 1.0000x reference)
"""FlowNet-C correlation layer (MAX_DISP=20, STRIDE=2) on 8 trn2 cores.

Strategy: shard by (batch b, output-row half). Core k handles b=k//2,
output rows [24*(k%2), 24*(k%2)+24). Contraction over C=128 runs on the
TensorEngine as banded-Gram matmuls: for each full-res row h and dy,
G = x1_row[128,96]^T @ x2p_rows[128,408] (3 dy values per matmul, f32r,
h-pair pooled via PSUM accumulation). The 41 correlation diagonals
corr[dx,w] = G[w, w+dx] are pulled out with shear-pattern DMAs (linear
element stride 409 walks partition+column simultaneously), split into
even/odd w so the 2x2 avg-pool finishes with one vector add.
Scale 1/(4*C) is folded into x1 on the host.
"""

import os

import numpy as np

import concourse.bacc as bacc
import concourse.bass as bass
import concourse.mybir as mybir
import concourse.tile as tile
from concourse.ap import AP
from concourse.bass import MemorySpace
from concourse.bass_utils import run_bass_kernel_spmd

MD = 20
K = 41
CC = K * K            # 1681
B, C, H, W = 4, 128, 96, 96
OH, OW = 48, 48
WP = W + 2 * MD       # 136
HH = 48               # full-res rows per core
NOH = 24              # output rows per core
ROWS = HH + K         # 89 x2p rows needed per core (dy up to 41 incl. garbage lane)
GW = 3 * WP           # 408: Gram tile free size (3 dy rows)

F32 = mybir.dt.float32
F32R = mybir.dt.float32r
BF16 = mybir.dt.bfloat16

LAST_EXEC_NS = None
_CACHED = None


def _build_nc():
    nc = bacc.Bacc("TRN2", target_bir_lowering=False)
    x1d = nc.dram_tensor("x1h", [C, HH * W], F32R, kind="ExternalInput")
    x2d = nc.dram_tensor("x2p", [C, ROWS * WP], F32R, kind="ExternalInput")
    outd = nc.dram_tensor("out", [NOH * OW, CC], F32, kind="ExternalOutput")

    with tile.TileContext(nc) as tc:
        with (
            tc.tile_pool(name="inp", bufs=1) as inp_pool,
            tc.tile_pool(name="gs", bufs=4) as gs_pool,
            tc.tile_pool(name="dd", bufs=8) as d_pool,
            tc.tile_pool(name="st", bufs=2) as s_pool,
            tc.tile_pool(name="ps", bufs=6, space=MemorySpace.PSUM) as psum_pool,
            tc.tile_pool(name="dr", bufs=6, space=MemorySpace.DRAM) as dram_pool,
        ):
            A = inp_pool.tile([C, HH * W], F32R)
            Bt = inp_pool.tile([C, ROWS * WP], F32R)
            # TRN2 ldweights encodes only ONE semaphore wait, so matmuls must
            # only ever depend on a single sem: the DVE engine counter (which
            # also guards PSUM-slot reuse).  Loads therefore bounce through
            # staging tiles and a DVE copy instead of DMAing into A/Bt
            # directly.
            with tc.tile_pool(name="stg", bufs=3) as stage_pool:
                for a0 in range(0, HH, 12):
                    stg = stage_pool.tile([C, 12 * W], F32R, tag="stg")
                    nc.sync.dma_start(stg[:], x1d[:, a0 * W:(a0 + 12) * W])
                    nc.vector.tensor_copy(A[:, a0 * W:(a0 + 12) * W], stg[:])
                for r0 in range(0, ROWS, 12):
                    r1 = min(r0 + 12, ROWS)
                    stg = stage_pool.tile([C, 12 * WP], F32R, tag="stg")
                    nc.sync.dma_start(stg[:, :(r1 - r0) * WP],
                                      x2d[:, r0 * WP:r1 * WP])
                    nc.vector.tensor_copy(Bt[:, r0 * WP:r1 * WP],
                                          stg[:, :(r1 - r0) * WP])

            for oh in range(NOH):
                h0 = 2 * oh
                S = s_pool.tile([OW, CC], F32)
                for s0 in range(0, 42, 9):          # supergroups of 3 dy-triples
                    groups = [g for g in range(s0, min(s0 + 9, 42), 3)]
                    pss = []
                    for g in groups:
                        ps = psum_pool.tile([W, GW], F32, tag="ps")
                        nc.tensor.matmul(
                            ps[:],
                            A[:, h0 * W:(h0 + 1) * W],
                            Bt[:, (h0 + g) * WP:(h0 + g) * WP + GW],
                            start=True, stop=False,
                        )
                        pss.append(ps)
                    for ps, g in zip(pss, groups):
                        nc.tensor.matmul(
                            ps[:],
                            A[:, (h0 + 1) * W:(h0 + 2) * W],
                            Bt[:, (h0 + 1 + g) * WP:(h0 + 1 + g) * WP + GW],
                            start=False, stop=True,
                        )
                    for ps, g in zip(pss, groups):
                        gs = gs_pool.tile([W, GW], BF16)
                        nc.vector.tensor_copy(gs[:], ps[:])
                        # bounce Gram through DRAM: shear APs are legal there
                        dscr = dram_pool.tile([W, GW], BF16)
                        nc.sync.dma_start(dscr[:], gs[:])
                        dt_ = dscr[:].tensor
                        for k3 in range(3):
                            dy = g + k3
                            if dy > 40:
                                continue
                            De = d_pool.tile([OW, K], BF16, tag="de")
                            Do = d_pool.tile([OW, K], BF16, tag="do")
                            # shear: G[w, w+dx] at flat idx w*(GW+1) + 136*k3 + dx
                            src_e = AP(dt_, 136 * k3, [[2 * (GW + 1), OW], [1, K]])
                            src_o = AP(dt_, 136 * k3 + GW + 1, [[2 * (GW + 1), OW], [1, K]])
                            nc.sync.dma_start(De[:], src_e)
                            nc.sync.dma_start(Do[:], src_o)
                            # 2x2-pool finish + write into staging at d = dx*41 + dy
                            nc.vector.tensor_add(S[:, dy::K], De[:], Do[:])
                nc.sync.dma_start(outd[oh * OW:(oh + 1) * OW, :], S[:])
    nc.compile()
    return nc


def kernel(x1: np.ndarray, x2: np.ndarray) -> np.ndarray:
    global LAST_EXEC_NS, _CACHED
    x1 = np.ascontiguousarray(np.asarray(x1, dtype=np.float32)) * np.float32(1.0 / (4 * C))
    x2 = np.asarray(x2, dtype=np.float32)
    x2p = np.zeros((B, C, HH + ROWS, WP), dtype=np.float32)
    x2p[:, :, MD:MD + H, MD:MD + W] = x2

    if _CACHED is None:
        _CACHED = _build_nc()
    nc = _CACHED

    in_maps = []
    for core in range(8):
        b, half = core // 2, core % 2
        a = np.ascontiguousarray(
            x1[b, :, half * HH:(half + 1) * HH, :].reshape(C, HH * W))
        x2s = np.ascontiguousarray(
            x2p[b, :, half * HH:half * HH + ROWS, :].reshape(C, ROWS * WP))
        in_maps.append({"x1h": a, "x2p": x2s})

    res = run_bass_kernel_spmd(
        nc, in_maps, core_ids=list(range(8)),
        trace=os.environ.get("CORR_TRACE") == "1",
    )
    LAST_EXEC_NS = res.exec_time_ns

    out = np.empty((B, CC, OH, OW), dtype=np.float32)
    for core in range(8):
        b, half = core // 2, core % 2
        r = res.results[core]["out"].reshape(NOH, OW, CC)
        out[b, :, half * NOH:(half + 1) * NOH, :] = r.transpose(2, 0, 1)
    return out

